# revision 11
# baseline (speedup 1.0000x reference)
"""Multi-head attention (B=8, T=2048, C=256, H=4) on 8 NeuronCores.

Data-parallel over batch: core b computes batch element b end-to-end.

Per-core dataflow (v3):
  xT   [C, T]      = PE-transpose of x (x pre-cast to bf16 on host)
  qkT  [2C, T]     = w_qk @ xT + b_qk   (bias via ScalarE Identity)
  v    [T, H, 65]  = x @ w_v.T + b_v    (ones column per head -> sumexp
                                         rides the PV matmul for free)
  attention, qt outer / head-pair inner / 16 k-chunks:
    scoresT[k,q] via K=64 matmuls in PE row groups 0/64 (pair overlaps)
    exp SPLIT across engines: ScalarE true Exp (9/16 chunks) and
      VectorE Schraudolph bit-trick (7/16): int16(round(s*A+B)) bitcast
      as bf16 == 2^(s*0.125/ln2) to +-3%; softmax averaging over 2048
      keys washes the error out (measured ~7e-4 total abs err).
    PV accumulates o2[h] = [65, 512] PSUM over chunks; row 64 = sumexp
    normalize per (qt,hp): DVE collects 2 sumexp rows, one batched
      reciprocal_approx_fast + bf16 cast, 2 row/col-tiled K=1 ones
      matmuls broadcast 1/sumexp into a [128,512] PSUM tile, ScalarE
      copies it to SBUF, one scalar_tensor_tensor per head fuses
      normalize+copy into yt (bf16).
  proj: out[t,:] = yT[:,t].T @ w_pT + b_p, bias on DVE, store to DRAM.

PSUM: sc pool 2x[128,1024] (scores/stageB/stageC/bc/proj staging) +
o2 pool 4x[65,512] (double-buffered across (qt,hp) parities) = 8 banks.
"""

import numpy as np
import ml_dtypes

import concourse.bass as bass
import concourse.tile as tile
from concourse import bacc, mybir
from concourse.bass_utils import run_bass_kernel_spmd
from concourse.masks import make_identity

B, T, C = 8, 2048, 256
H, HD = 4, 64
N_CORES = 8
F32 = mybir.dt.float32
F32R = mybir.dt.float32r
BF16 = mybir.dt.bfloat16
I16 = mybir.dt.int16

QT = 512                # q-tile (columns per score matmul)
NQT = T // QT           # 4
KC = T // 128           # 16 k-chunks of 128

# Schraudolph exp for bf16-bitcast: bf16(i16) ~= 2^(i16/128 - 127)
# want exp(s*0.125) = 2^(s*0.125/ln2):  i16 = s*(16/ln2) + 127*128 - c
EXP_A = float(np.float32(128.0 * 0.125 / np.log(2.0)))
EXP_B = float(np.float32(16256.0 - 7.5))

# k-chunks whose exp runs as Schraudolph on the DVE (rest: ScalarE Exp)
DVE_CHUNKS = frozenset((1, 3, 5, 7, 9, 11, 13))


def build_nc():
    nc = bacc.Bacc("TRN2", target_bir_lowering=False, debug=False,
                   num_devices=N_CORES)

    x_ap = nc.dram_tensor("xbf", [T, C], BF16, kind="ExternalInput").ap()
    wqk_ap = nc.dram_tensor("w_qkT", [C, 2 * C], F32R, kind="ExternalInput").ap()
    wv_ap = nc.dram_tensor("w_vT", [C, C], F32R, kind="ExternalInput").ap()
    wp_ap = nc.dram_tensor("w_pT", [C, C], F32R, kind="ExternalInput").ap()
    bqk_ap = nc.dram_tensor("b_qk", [4, 128], F32, kind="ExternalInput").ap()
    bv_ap = nc.dram_tensor("b_v", [C], F32, kind="ExternalInput").ap()
    bp_ap = nc.dram_tensor("b_p", [C], F32, kind="ExternalInput").ap()
    out_ap = nc.dram_tensor("out", [T, C], F32, kind="ExternalOutput").ap()

    with tile.TileContext(nc) as tc:
        with (
            tc.tile_pool(name="consts", bufs=1) as consts,
            tc.tile_pool(name="xstage", bufs=4) as xstage,
            tc.tile_pool(name="xt", bufs=1) as xtp,
            tc.tile_pool(name="qkt", bufs=1) as qktp,
            tc.tile_pool(name="vsb", bufs=1) as vsbp,
            tc.tile_pool(name="expa", bufs=4) as expa,
            tc.tile_pool(name="expd", bufs=4) as expd,
            tc.tile_pool(name="yt", bufs=1) as ytp,
            tc.tile_pool(name="small", bufs=2) as small,
            tc.tile_pool(name="ostage", bufs=4) as ostage,
            tc.tile_pool(name="scps", bufs=4, space="PSUM") as scps,
            tc.tile_pool(name="o2ps", bufs=2, space="PSUM") as o2ps,
        ):
            # ---- stage A load: x first (critical path), via sync HWDGE ---
            x_re = x_ap.rearrange("(b a p) c -> b p a c", b=4, p=128)
            xsbig = [None] * 4
            for b in range(4):
                xsbig[b] = xstage.tile([128, 4, C], BF16, tag="xs", name=f"xs{b}")
                nc.sync.dma_start(xsbig[b][:], x_re[b])

            # ---- constants / weights -------------------------------------
            ident = consts.tile([128, 128], BF16, tag="ident")
            make_identity(nc, ident[:])
            ones_bc = consts.tile([33, 64], BF16, tag="ones_bc")
            nc.vector.memset(ones_bc[:], 1.0)
            onescol = consts.tile([128, H], BF16, tag="onescol")
            nc.vector.memset(onescol[:], 1.0)

            w_qk = [consts.tile([128, 2 * C], BF16, tag=f"wqk{c}", name=f"wqk{c}") for c in range(2)]
            for c in range(2):
                nc.gpsimd.dma_start(w_qk[c][:], wqk_ap[128 * c:128 * (c + 1), :])
            w_v = [consts.tile([128, C], BF16, tag=f"wv{c}", name=f"wv{c}") for c in range(2)]
            for c in range(2):
                nc.gpsimd.dma_start(w_v[c][:], wv_ap[128 * c:128 * (c + 1), :])
            w_p = [consts.tile([128, C], BF16, tag=f"wp{c}", name=f"wp{c}") for c in range(2)]
            for c in range(2):
                nc.gpsimd.dma_start(w_p[c][:], wp_ap[128 * c:128 * (c + 1), :])

            b_qk = consts.tile([128, 4], F32, tag="bqk")
            nc.gpsimd.dma_start(b_qk[:], bqk_ap.rearrange("c p -> p c"))
            b_p = consts.tile([128, C], F32, tag="bp")
            bp_bc = bass.AP(tensor=bp_ap.tensor, offset=bp_ap.offset,
                            ap=[[0, 128]] + list(bp_ap.ap))
            nc.gpsimd.dma_start(b_p[:], bp_bc)
            b_v = consts.tile([128, C], F32, tag="bv")
            bv_bc = bass.AP(tensor=bv_ap.tensor, offset=bv_ap.offset,
                            ap=[[0, 128]] + list(bv_ap.ap))
            nc.gpsimd.dma_start(b_v[:], bv_bc)

            # ---- stage A: PE-transpose to xT -----------------------------
            xt = [xtp.tile([128, T], BF16, tag=f"xt{c}", name=f"xt{c}") for c in range(2)]
            for tt in range(KC):
                xs = xsbig[tt // 4][:, tt % 4, :]
                for c in range(2):
                    if c == 0:
                        ps = scps.tile([128, 128], BF16, tag="sc", name="tp0")
                    else:
                        ps = o2ps.tile([128, 128], BF16, tag=f"o2h{tt % 2}",
                                       name=f"tp{tt % 2}")
                    nc.tensor.transpose(ps[:], xs[:, 128 * c:128 * (c + 1)], ident[:])
                    nc.vector.tensor_copy(xt[c][:, 128 * tt:128 * (tt + 1)], ps[:])

            # ---- stage B: qkT [2C, T] = w_qk.T @ xT + b_qk ---------------
            # m-outer, n-inner with 4 live PSUM halves: the w_qk stationary
            # is loaded once per (m, c) instead of per (m, n, c).
            qkt = [qktp.tile([128, T], BF16, tag=f"qkt{m}", name=f"qkt{m}") for m in range(4)]
            def stage_b(m):
                pss = [scps.tile([128, QT], F32, tag="sc", name=f"bps{m}{j}")
                       for j in range(NQT)]
                for c in range(2):
                    for n in range(NQT):
                        nc.tensor.matmul(
                            pss[n][:], w_qk[c][:, 128 * m:128 * (m + 1)],
                            xt[c][:, QT * n:QT * (n + 1)],
                            start=(c == 0), stop=(c == 1))
                for n in range(NQT):
                    nc.scalar.add(
                        qkt[m][:, QT * n:QT * (n + 1)], pss[n][:],
                        b_qk[:, m:m + 1])

            stage_b(2)
            stage_b(0)

            # ---- stage C: v [T, H, 65] natural + bias + ones column ------
            vsb = [vsbp.tile([128, H, HD + 1], BF16, tag=f"v{tt}", name=f"v{tt}") for tt in range(KC)]
            for tt in range(KC):
                ps = scps.tile([128, QT], F32, tag="sc", name="cps")
                for c in range(2):
                    nc.tensor.matmul(
                        ps[:, 0:C], xt[c][:, 128 * tt:128 * (tt + 1)], w_v[c][:],
                        start=(c == 0), stop=(c == 1))
                nc.vector.tensor_add(
                    vsb[tt][:, :, 0:HD],
                    ps[:, 0:C].rearrange("p (h d) -> p h d", h=H),
                    b_v[:].rearrange("p (h d) -> p h d", h=H))
                nc.vector.tensor_copy(
                    vsb[tt][:, :, HD:HD + 1],
                    onescol[:].rearrange("p (h o) -> p h o", o=1))

            stage_b(3)
            stage_b(1)

            # ---- stage D: attention, qt outer / head-pair / k-chunks -----
            yt = [ytp.tile([128, T], BF16, tag=f"yt{hp}", name=f"yt{hp}") for hp in range(2)]
            for qt in range(NQT):
                for hp in range(2):
                    qT = qkt[hp]
                    kT = qkt[hp + 2]
                    o2 = [o2ps.tile([HD + 1, QT], F32, tag=f"o2h{h}",
                                    name=f"o2{h}") for h in range(2)]
                    for i in range(KC):
                        scs = [scps.tile([128, QT], F32, tag="sc", name=f"sc{h}")
                               for h in range(2)]
                        for h in range(2):
                            nc.tensor.matmul(
                                scs[h][:],
                                kT[64 * h:64 * (h + 1), 128 * i:128 * (i + 1)],
                                qT[64 * h:64 * (h + 1), QT * qt:QT * (qt + 1)],
                                start=True, stop=True)
                        exs = []
                        for h in range(2):
                            if i in DVE_CHUNKS:
                                exd = expd.tile([128, QT], I16, tag="exd")
                                nc.vector.tensor_scalar(
                                    exd[:], scs[h][:], EXP_A, EXP_B,
                                    mybir.AluOpType.mult, mybir.AluOpType.add)
                                exs.append(exd[:].bitcast(BF16))
                            else:
                                exa = expa.tile([128, QT], BF16, tag="exa")
                                nc.scalar.activation(
                                    exa[:], scs[h][:],
                                    mybir.ActivationFunctionType.Exp,
                                    bias=0.0, scale=0.125)
                                exs.append(exa[:])
                        for h in range(2):
                            nc.tensor.matmul(
                                o2[h][:],
                                vsb[i][:, 2 * hp + h, :],
                                exs[h][:],
                                start=(i == 0), stop=(i == KC - 1))
                    # ---- normalize + copy to yt --------------------------
                    se = small.tile([33, QT], F32, tag="se")
                    for h in range(2):
                        nc.vector.tensor_copy(
                            se[32 * h:32 * h + 1, :], o2[h][HD:HD + 1, :])
                    rec_f = small.tile([33, QT], F32, tag="rec_f")
                    nc.vector.reciprocal_approx_fast(rec_f[:], se[:])
                    rec = small.tile([33, QT], BF16, tag="rec")
                    nc.vector.tensor_copy(rec[:], rec_f[:])
                    bc = scps.tile([128, QT], F32, tag="sc", name="bc")
                    for h in range(2):
                        nc.tensor.matmul(
                            bc[64 * h:64 * (h + 1), :],
                            ones_bc[32 * h:32 * h + 1, :],
                            rec[32 * h:32 * h + 1, :],
                            start=True, stop=True,
                            tile_position=(32 * h, 64 * h))
                    bcs = small.tile([128, QT], BF16, tag="bcs")
                    nc.scalar.copy(bcs[:], bc[:])
                    for h in range(2):
                        nc.vector.scalar_tensor_tensor(
                            yt[hp][64 * h:64 * (h + 1), QT * qt:QT * (qt + 1)],
                            o2[h][0:HD, :], 1.0, bcs[64 * h:64 * (h + 1), :],
                            mybir.AluOpType.mult, mybir.AluOpType.mult)
                # ---- proj for this q-tile --------------------------------
                for tt in range(qt * QT // 128, (qt + 1) * QT // 128):
                    ps = scps.tile([128, QT], F32, tag="sc", name="pps")
                    for c in range(2):
                        nc.tensor.matmul(
                            ps[:, 0:C], yt[c][:, 128 * tt:128 * (tt + 1)], w_p[c][:],
                            start=(c == 0), stop=(c == 1))
                    ost = ostage.tile([128, C], F32, tag="ost")
                    nc.vector.tensor_add(ost[:], ps[:, 0:C], b_p[:])
                    nc.sync.dma_start(out_ap[128 * tt:128 * (tt + 1), :], ost[:])
    nc.compile()
    return nc


_NC_CACHE = []


def _get_nc():
    if not _NC_CACHE:
        _NC_CACHE.append(build_nc())
    return _NC_CACHE[0]


def make_in_maps(x, w_qkv, b_qkv, w_proj, b_proj):
    shared = {
        "w_qkT": np.ascontiguousarray(w_qkv[:2 * C].T, dtype=np.float32),
        "w_vT": np.ascontiguousarray(w_qkv[2 * C:].T, dtype=np.float32),
        "w_pT": np.ascontiguousarray(w_proj.T, dtype=np.float32),
        "b_qk": np.ascontiguousarray(b_qkv[:2 * C].reshape(4, 128), dtype=np.float32),
        "b_v": np.ascontiguousarray(b_qkv[2 * C:], dtype=np.float32),
        "b_p": np.ascontiguousarray(b_proj, dtype=np.float32),
    }
    xbf = np.asarray(x, dtype=np.float32).astype(ml_dtypes.bfloat16)
    return [dict(shared, xbf=np.ascontiguousarray(xbf[b])) for b in range(B)]


def run(x, w_qkv, b_qkv, w_proj, b_proj, trace=False):
    nc = _get_nc()
    in_maps = make_in_maps(np.asarray(x), np.asarray(w_qkv), np.asarray(b_qkv),
                           np.asarray(w_proj), np.asarray(b_proj))
    res = run_bass_kernel_spmd(nc, in_maps, list(range(N_CORES)), trace=trace)
    out = np.stack([res.results[b]["out"] for b in range(B)])
    return out, res


def kernel(x, w_qkv, b_qkv, w_proj, b_proj):
    out, _ = run(x, w_qkv, b_qkv, w_proj, b_proj, trace=False)
    return out


# revision 12
# speedup vs baseline: 1.2052x; 1.2052x over previous
"""Multi-head attention (B=8, T=2048, C=256, H=4) on 8 NeuronCores.

Data-parallel over batch: core b computes batch element b end-to-end.

Per-core dataflow (v3):
  xT   [C, T]      = PE-transpose of x (x pre-cast to bf16 on host)
  qkT  [2C, T]     = w_qk @ xT + b_qk   (bias via ScalarE Identity)
  v    [T, H, 65]  = x @ w_v.T + b_v    (ones column per head -> sumexp
                                         rides the PV matmul for free)
  attention, qt outer / head-pair inner / 16 k-chunks:
    scoresT[k,q] via K=64 matmuls in PE row groups 0/64 (pair overlaps)
    exp SPLIT across engines: ScalarE true Exp (9/16 chunks) and
      VectorE Schraudolph bit-trick (7/16): int16(round(s*A+B)) bitcast
      as bf16 == 2^(s*0.125/ln2) to +-3%; softmax averaging over 2048
      keys washes the error out (measured ~7e-4 total abs err).
    PV accumulates o2[h] = [65, 512] PSUM over chunks; row 64 = sumexp
    normalize per (qt,hp): DVE collects 2 sumexp rows, one batched
      reciprocal_approx_fast + bf16 cast, 2 row/col-tiled K=1 ones
      matmuls broadcast 1/sumexp into a [128,512] PSUM tile, ScalarE
      copies it to SBUF, one scalar_tensor_tensor per head fuses
      normalize+copy into yt (bf16).
  proj: out[t,:] = yT[:,t].T @ w_pT + b_p, bias on DVE, store to DRAM.

PSUM: sc pool 2x[128,1024] (scores/stageB/stageC/bc/proj staging) +
o2 pool 4x[65,512] (double-buffered across (qt,hp) parities) = 8 banks.
"""

import numpy as np
import ml_dtypes

import concourse.bass as bass
import concourse.tile as tile
from concourse import bacc, mybir
from concourse.bass_utils import run_bass_kernel_spmd
from concourse.masks import make_identity

B, T, C = 8, 2048, 256
H, HD = 4, 64
N_CORES = 8
F32 = mybir.dt.float32
F32R = mybir.dt.float32r
BF16 = mybir.dt.bfloat16
I16 = mybir.dt.int16

QT = 512                # q-tile (columns per score matmul)
NQT = T // QT           # 4
KC = T // 128           # 16 k-chunks of 128

# Schraudolph exp for bf16-bitcast: bf16(i16) ~= 2^(i16/128 - 127)
# want exp(s*0.125) = 2^(s*0.125/ln2):  i16 = s*(16/ln2) + 127*128 - c
EXP_A = float(np.float32(128.0 * 0.125 / np.log(2.0)))
EXP_B = float(np.float32(16256.0 - 7.5))

# k-chunks whose exp runs as Schraudolph on the DVE (rest: ScalarE Exp)
DVE_CHUNKS = frozenset((1, 3, 5, 7, 9, 11, 13))


def build_nc():
    nc = bacc.Bacc("TRN2", target_bir_lowering=False, debug=False,
                   num_devices=N_CORES)

    x_ap = nc.dram_tensor("xbf", [T, C], BF16, kind="ExternalInput").ap()
    wqk_ap = nc.dram_tensor("w_qkT", [C, 2 * C], F32R, kind="ExternalInput").ap()
    wv_ap = nc.dram_tensor("w_vT", [C, C], F32R, kind="ExternalInput").ap()
    wp_ap = nc.dram_tensor("w_pT", [C, C], F32R, kind="ExternalInput").ap()
    bqk_ap = nc.dram_tensor("b_qk", [4, 128], F32, kind="ExternalInput").ap()
    bv_ap = nc.dram_tensor("b_v", [C], F32, kind="ExternalInput").ap()
    bp_ap = nc.dram_tensor("b_p", [C], F32, kind="ExternalInput").ap()
    out_ap = nc.dram_tensor("out", [T, C], F32, kind="ExternalOutput").ap()

    with tile.TileContext(nc) as tc:
        with (
            tc.tile_pool(name="consts", bufs=1) as consts,
            tc.tile_pool(name="xstage", bufs=4) as xstage,
            tc.tile_pool(name="xt", bufs=1) as xtp,
            tc.tile_pool(name="qkt", bufs=1) as qktp,
            tc.tile_pool(name="vsb", bufs=1) as vsbp,
            tc.tile_pool(name="expa", bufs=4) as expa,
            tc.tile_pool(name="expd", bufs=4) as expd,
            tc.tile_pool(name="yt", bufs=1) as ytp,
            tc.tile_pool(name="small", bufs=2) as small,
            tc.tile_pool(name="ostage", bufs=4) as ostage,
            tc.tile_pool(name="scps", bufs=4, space="PSUM") as scps,
            tc.tile_pool(name="o2ps", bufs=2, space="PSUM") as o2ps,
        ):
            # ---- stage A load: x first (critical path), via sync HWDGE ---
            x_re = x_ap.rearrange("(b a p) c -> b p a c", b=4, p=128)
            xsbig = [None] * 4
            for b in range(4):
                xsbig[b] = xstage.tile([128, 4, C], BF16, tag="xs", name=f"xs{b}")
                nc.sync.dma_start(xsbig[b][:], x_re[b])

            # ---- constants / weights -------------------------------------
            ident = consts.tile([128, 128], BF16, tag="ident")
            make_identity(nc, ident[:])
            ones_bc = consts.tile([33, 64], BF16, tag="ones_bc")
            nc.vector.memset(ones_bc[:], 1.0)
            onescol = consts.tile([128, H], BF16, tag="onescol")
            nc.vector.memset(onescol[:], 1.0)

            w_qk = [consts.tile([128, 2 * C], BF16, tag=f"wqk{c}", name=f"wqk{c}") for c in range(2)]
            for c in range(2):
                nc.gpsimd.dma_start(w_qk[c][:], wqk_ap[128 * c:128 * (c + 1), :])
            w_v = [consts.tile([128, C], BF16, tag=f"wv{c}", name=f"wv{c}") for c in range(2)]
            for c in range(2):
                nc.gpsimd.dma_start(w_v[c][:], wv_ap[128 * c:128 * (c + 1), :])
            w_p = [consts.tile([128, C], BF16, tag=f"wp{c}", name=f"wp{c}") for c in range(2)]
            for c in range(2):
                nc.gpsimd.dma_start(w_p[c][:], wp_ap[128 * c:128 * (c + 1), :])

            b_qk = consts.tile([128, 4], F32, tag="bqk")
            nc.gpsimd.dma_start(b_qk[:], bqk_ap.rearrange("c p -> p c"))
            b_p = consts.tile([128, C], F32, tag="bp")
            bp_bc = bass.AP(tensor=bp_ap.tensor, offset=bp_ap.offset,
                            ap=[[0, 128]] + list(bp_ap.ap))
            nc.gpsimd.dma_start(b_p[:], bp_bc)
            b_v = consts.tile([128, C], F32, tag="bv")
            bv_bc = bass.AP(tensor=bv_ap.tensor, offset=bv_ap.offset,
                            ap=[[0, 128]] + list(bv_ap.ap))
            nc.gpsimd.dma_start(b_v[:], bv_bc)

            # ---- stage A: PE-transpose to xT -----------------------------
            xt = [xtp.tile([128, T], BF16, tag=f"xt{c}", name=f"xt{c}") for c in range(2)]
            for tt in range(KC):
                xs = xsbig[tt // 4][:, tt % 4, :]
                for c in range(2):
                    if c == 0:
                        ps = scps.tile([128, 128], BF16, tag="sc", name="tp0")
                    else:
                        ps = o2ps.tile([128, 128], BF16, tag=f"o2h{tt % 2}",
                                       name=f"tp{tt % 2}")
                    nc.tensor.transpose(ps[:], xs[:, 128 * c:128 * (c + 1)], ident[:])
                    nc.vector.tensor_copy(xt[c][:, 128 * tt:128 * (tt + 1)], ps[:])

            # ---- stage B: qkT [2C, T] = w_qk.T @ xT + b_qk ---------------
            # m-outer, n-inner with 4 live PSUM halves: the w_qk stationary
            # is loaded once per (m, c) instead of per (m, n, c).
            qkt = [qktp.tile([128, T], BF16, tag=f"qkt{m}", name=f"qkt{m}") for m in range(4)]
            def stage_b(m):
                pss = [scps.tile([128, QT], F32, tag="sc", name=f"bps{m}{j}")
                       for j in range(NQT)]
                for c in range(2):
                    for n in range(NQT):
                        nc.tensor.matmul(
                            pss[n][:], w_qk[c][:, 128 * m:128 * (m + 1)],
                            xt[c][:, QT * n:QT * (n + 1)],
                            start=(c == 0), stop=(c == 1))
                for n in range(NQT):
                    nc.scalar.add(
                        qkt[m][:, QT * n:QT * (n + 1)], pss[n][:],
                        b_qk[:, m:m + 1])

            stage_b(2)
            stage_b(0)

            # ---- stage C: v [T, H, 65] natural + bias + ones column ------
            vsb = [vsbp.tile([128, H, HD + 1], BF16, tag=f"v{tt}", name=f"v{tt}") for tt in range(KC)]
            for tt in range(KC):
                ps = scps.tile([128, QT], F32, tag="sc", name="cps")
                for c in range(2):
                    nc.tensor.matmul(
                        ps[:, 0:C], xt[c][:, 128 * tt:128 * (tt + 1)], w_v[c][:],
                        start=(c == 0), stop=(c == 1))
                nc.vector.tensor_add(
                    vsb[tt][:, :, 0:HD],
                    ps[:, 0:C].rearrange("p (h d) -> p h d", h=H),
                    b_v[:].rearrange("p (h d) -> p h d", h=H))
                nc.vector.tensor_copy(
                    vsb[tt][:, :, HD:HD + 1],
                    onescol[:].rearrange("p (h o) -> p h o", o=1))

            stage_b(3)
            stage_b(1)

            # ---- stage D: attention, qt outer / head-pair / k-chunks -----
            yt = [ytp.tile([128, T], BF16, tag=f"yt{hp}", name=f"yt{hp}") for hp in range(2)]
            # Normalize (bc matmul / ScalarE copy / STT) and proj are
            # DEFERRED into the middle of the NEXT block so the in-order
            # PE queue never stalls on the DVE recip chain.
            def make_norm(qt, hp, o2, rec):
                def norm():
                    bc = scps.tile([128, QT], F32, tag="sc", name="bc")
                    for h in range(2):
                        nc.tensor.matmul(
                            bc[64 * h:64 * (h + 1), :],
                            ones_bc[32 * h:32 * h + 1, :],
                            rec[32 * h:32 * h + 1, :],
                            start=True, stop=True,
                            tile_position=(32 * h, 64 * h))
                    bcs = small.tile([128, QT], BF16, tag="bcs")
                    nc.scalar.copy(bcs[:], bc[:])
                    for h in range(2):
                        nc.vector.scalar_tensor_tensor(
                            yt[hp][64 * h:64 * (h + 1), QT * qt:QT * (qt + 1)],
                            o2[h][0:HD, :], 1.0, bcs[64 * h:64 * (h + 1), :],
                            mybir.AluOpType.mult, mybir.AluOpType.mult)
                return norm

            def make_proj(qt):
                def proj():
                    for tt in range(qt * QT // 128, (qt + 1) * QT // 128):
                        ps = scps.tile([128, QT], F32, tag="sc", name="pps")
                        for c in range(2):
                            nc.tensor.matmul(
                                ps[:, 0:C], yt[c][:, 128 * tt:128 * (tt + 1)],
                                w_p[c][:], start=(c == 0), stop=(c == 1))
                        ost = ostage.tile([128, C], F32, tag="ost")
                        nc.vector.tensor_add(ost[:], ps[:, 0:C], b_p[:])
                        nc.sync.dma_start(out_ap[128 * tt:128 * (tt + 1), :], ost[:])
                return proj

            pend_norm, pend_proj = None, None
            for qt in range(NQT):
                for hp in range(2):
                    qT = qkt[hp]
                    kT = qkt[hp + 2]
                    o2 = [o2ps.tile([HD + 1, QT], F32, tag=f"o2h{h}",
                                    name=f"o2{h}") for h in range(2)]
                    for i in range(KC):
                        if i == 3 and pend_norm is not None:
                            pend_norm()
                            pend_norm = None
                        if i == 6 and pend_proj is not None:
                            pend_proj()
                            pend_proj = None
                        scs = [scps.tile([128, QT], F32, tag="sc", name=f"sc{h}")
                               for h in range(2)]
                        for h in range(2):
                            nc.tensor.matmul(
                                scs[h][:],
                                kT[64 * h:64 * (h + 1), 128 * i:128 * (i + 1)],
                                qT[64 * h:64 * (h + 1), QT * qt:QT * (qt + 1)],
                                start=True, stop=True)
                        exs = []
                        for h in range(2):
                            if i in DVE_CHUNKS:
                                exd = expd.tile([128, QT], I16, tag="exd")
                                nc.vector.tensor_scalar(
                                    exd[:], scs[h][:], EXP_A, EXP_B,
                                    mybir.AluOpType.mult, mybir.AluOpType.add)
                                exs.append(exd[:].bitcast(BF16))
                            else:
                                exa = expa.tile([128, QT], BF16, tag="exa")
                                nc.scalar.activation(
                                    exa[:], scs[h][:],
                                    mybir.ActivationFunctionType.Exp,
                                    bias=0.0, scale=0.125)
                                exs.append(exa[:])
                        for h in range(2):
                            nc.tensor.matmul(
                                o2[h][:],
                                vsb[i][:, 2 * hp + h, :],
                                exs[h][:],
                                start=(i == 0), stop=(i == KC - 1))
                    # sumexp -> 1/sumexp on the DVE (fills its idle tail)
                    se = small.tile([33, QT], F32, tag="se")
                    for h in range(2):
                        nc.vector.tensor_copy(
                            se[32 * h:32 * h + 1, :], o2[h][HD:HD + 1, :])
                    rec_f = small.tile([33, QT], F32, tag="rec_f")
                    nc.vector.reciprocal_approx_fast(rec_f[:], se[:])
                    rec = small.tile([33, QT], BF16, tag="rec")
                    nc.vector.tensor_copy(rec[:], rec_f[:])
                    pend_norm = make_norm(qt, hp, o2, rec)
                if qt > 0:
                    pend_proj = make_proj(qt - 1)
            pend_norm()
            pend_proj()
            make_proj(NQT - 1)()
    nc.compile()
    return nc


_NC_CACHE = []


def _get_nc():
    if not _NC_CACHE:
        _NC_CACHE.append(build_nc())
    return _NC_CACHE[0]


def make_in_maps(x, w_qkv, b_qkv, w_proj, b_proj):
    shared = {
        "w_qkT": np.ascontiguousarray(w_qkv[:2 * C].T, dtype=np.float32),
        "w_vT": np.ascontiguousarray(w_qkv[2 * C:].T, dtype=np.float32),
        "w_pT": np.ascontiguousarray(w_proj.T, dtype=np.float32),
        "b_qk": np.ascontiguousarray(b_qkv[:2 * C].reshape(4, 128), dtype=np.float32),
        "b_v": np.ascontiguousarray(b_qkv[2 * C:], dtype=np.float32),
        "b_p": np.ascontiguousarray(b_proj, dtype=np.float32),
    }
    xbf = np.asarray(x, dtype=np.float32).astype(ml_dtypes.bfloat16)
    return [dict(shared, xbf=np.ascontiguousarray(xbf[b])) for b in range(B)]


def run(x, w_qkv, b_qkv, w_proj, b_proj, trace=False):
    nc = _get_nc()
    in_maps = make_in_maps(np.asarray(x), np.asarray(w_qkv), np.asarray(b_qkv),
                           np.asarray(w_proj), np.asarray(b_proj))
    res = run_bass_kernel_spmd(nc, in_maps, list(range(N_CORES)), trace=trace)
    out = np.stack([res.results[b]["out"] for b in range(B)])
    return out, res


def kernel(x, w_qkv, b_qkv, w_proj, b_proj):
    out, _ = run(x, w_qkv, b_qkv, w_proj, b_proj, trace=False)
    return out
